# revision 8
# baseline (speedup 1.0000x reference)
"""Bidirectional masked-LSTM encoder on 8 Trainium2 NeuronCores.

Sharding: 2 directions x 4 batch-quarters = 8 cores, one uniform SPMD
program. Each core computes the input projection xz = [x,1] @ [W;b] for
its 8 sequences (token-batched matmuls, bf16), then runs the sequential
512-step recurrence (h @ U streamed through the PE each step). The
backward direction is realized by feeding time-reversed x/mask to cores
4-7 and un-reversing on the host.
"""

import numpy as np
import ml_dtypes

import concourse.bass as bass
import concourse.mybir as mybir
import concourse.tile as tile
from concourse.bass import ds, ts
from concourse.bass_utils import run_bass_kernel_spmd

BF16 = mybir.dt.bfloat16
F32 = mybir.dt.float32
AF = mybir.ActivationFunctionType

B, S, H = 32, 512, 1024
BL = 8          # sequences per core
G4 = 4 * H      # 4096 gate columns
KA = 9          # K-tiles for augmented H (1024 + bias row -> 1152)
HA = KA * 128
NB = BL * S     # tokens per core


def _split_waits(nc, cap=1):
    # walrus in this container rejects >1 sync wait per instruction;
    # hoist excess waits onto preceding same-engine drains.
    n_new = 0
    for f in nc.m.functions:
        for bb in f.blocks:
            insts = bb.instructions
            out = []
            for ins in insts:
                si = getattr(ins, "sync_info", None)
                w = list(si.on_wait) if (si is not None and si.on_wait) else []
                if len(w) > cap:
                    excess, keep = w[:-cap], w[-cap:]
                    for j in range(0, len(excess), cap):
                        n_new += 1
                        out.append(mybir.InstDrain(
                            name=f"{ins.name}-ws{j}", engine=ins.engine,
                            ins=[], outs=[],
                            sync_info=mybir.SyncInfo(
                                on_wait=excess[j:j + cap], on_update=[]),
                        ))
                    si.on_wait = keep
                out.append(ins)
            if len(out) != len(insts):
                bb.instructions = out
    return n_new


def _build():
    nc = bass.Bass(trn_type="TRN2")
    xT = nc.dram_tensor("xT", [HA, NB], BF16, kind="ExternalInput")
    Wt = nc.dram_tensor("Wt", [HA, G4], BF16, kind="ExternalInput")
    Ut = nc.dram_tensor("Ut", [H, G4], BF16, kind="ExternalInput")
    mk = nc.dram_tensor("mk", [BL, S], F32, kind="ExternalInput")
    id8 = nc.dram_tensor("id8", [BL, BL], BF16, kind="ExternalInput")
    oenc = nc.dram_tensor("oenc", [S, BL, H], F32, kind="ExternalOutput")
    ohc = nc.dram_tensor("ohc", [2, BL, H], F32, kind="ExternalOutput")
    xz = nc.dram_tensor("xz", [S, BL, G4], BF16, kind="Internal")

    with tile.TileContext(nc) as tc:
        with (
            tc.tile_pool(name="big", bufs=1) as big,      # xT then U (shared slot)
            tc.tile_pool(name="wn", bufs=2) as wn,        # W column-chunk
            tc.tile_pool(name="cst", bufs=1) as cst,      # constants + state
            tc.tile_pool(name="xzo", bufs=3) as xzo,      # phase-A psum->bf16 out
            tc.tile_pool(name="xzt", bufs=2) as xzt,      # phase-B xz[s]
            tc.tile_pool(name="gat", bufs=2) as gat,      # gates
            tc.tile_pool(name="tmp", bufs=2) as tmp,
            tc.tile_pool(name="mb", bufs=2) as mbp,
            tc.tile_pool(name="psA", bufs=2, space="PSUM") as psA,
            tc.tile_pool(name="psz", bufs=4, space="PSUM") as psz,
            tc.tile_pool(name="ptp", bufs=2, space="PSUM") as ptp,
        ):
            id8_sb = cst.tile([BL, BL], BF16, tag="id8")
            nc.sync.dma_start(id8_sb[:, :], id8[:, :])
            mk_sb = cst.tile([BL, S], F32, tag="mk")
            nc.sync.dma_start(mk_sb[:, :], mk[:, :])
            ones = cst.tile([BL, H], F32, tag="ones")
            nc.vector.memset(ones[:, :], 1.0)
            h_st = cst.tile([BL, H], F32, tag="h")
            c_st = cst.tile([BL, H], F32, tag="c")
            hbf = cst.tile([BL, H], BF16, tag="hbf")
            hT = cst.tile([128, 8 * BL], BF16, tag="hT")
            nc.vector.memset(h_st[:, :], 0.0)
            nc.vector.memset(c_st[:, :], 0.0)
            nc.vector.memset(hbf[:, :], 0.0)
            nc.vector.memset(hT[:, :], 0.0)

            # ---- Phase A: xz[s, b, :] = [x; 1] @ [W; b]  (token-batched) ----
            xT_sb = big.tile([128, KA * NB], BF16, tag="big")
            nc.sync.dma_start(
                xT_sb[:, :].rearrange("p (k t) -> p k t", k=KA),
                xT.rearrange("(k p) t -> p k t", p=128))
            for n in range(8):  # 512-col chunks of G4
                wt = wn.tile([128, KA * 512], BF16)
                nc.sync.dma_start(
                    wt[:, :].rearrange("p (k c) -> p k c", k=KA),
                    Wt[:, n * 512:(n + 1) * 512]
                    .rearrange("(k p) c -> p k c", p=128))
                for m in range(32):  # 128-token tiles (b = m//4, s0 = m%4*128)
                    ps = psA.tile([128, 512], F32)
                    for k in range(KA):
                        nc.tensor.matmul(
                            ps[:, :],
                            xT_sb[:, k * NB + m * 128: k * NB + (m + 1) * 128],
                            wt[:, ts(k, 512)],
                            start=(k == 0), stop=(k == KA - 1))
                    xb = xzo.tile([128, 512], BF16)
                    nc.vector.tensor_copy(xb[:, :], ps[:, :])
                    b_i, s0 = m // 4, (m % 4) * 128
                    nc.sync.dma_start(
                        xz[s0:s0 + 128, b_i, n * 512:(n + 1) * 512],
                        xb[:, :])

            # ---- Phase B: the recurrence ----
            u_sb = big.tile([128, 8 * G4], BF16, tag="big")
            nc.sync.dma_start(
                u_sb[:, :].rearrange("p (k n) -> p k n", k=8),
                Ut.rearrange("(k p) n -> p k n", p=128))

            with tc.For_i(0, S, 1) as i:
                xt = xzt.tile([BL, G4], BF16)
                nc.sync.dma_start(
                    xt[:, :],
                    xz[ds(i, 1), :, :].rearrange("o b c -> (o b) c"))
                gt = gat.tile([BL, G4], F32)
                for c in range(8):  # gate chunk: 0,1=i 2,3=f 4,5=g 6,7=o
                    ps = psz.tile([BL, 512], F32)
                    nc.tensor.matmul(  # psum <- xz (identity over batch)
                        ps[:, :], id8_sb[:, :], xt[:, ts(c, 512)],
                        start=True, stop=False)
                    for k in range(8):
                        nc.tensor.matmul(
                            ps[:, :], hT[:, ts(k, BL)],
                            u_sb[:, k * G4 + c * 512: k * G4 + (c + 1) * 512],
                            start=False, stop=(k == 7))
                    func = AF.Tanh if c in (4, 5) else AF.Sigmoid
                    nc.scalar.activation(gt[:, ts(c, 512)], ps[:, :], func)

                t_ig = tmp.tile([BL, H], F32, tag="t1")
                t_fc = tmp.tile([BL, H], F32, tag="t2")
                nc.vector.tensor_mul(t_ig[:, :], gt[:, 0:H], gt[:, 2 * H:3 * H])
                nc.vector.tensor_mul(t_fc[:, :], gt[:, H:2 * H], c_st[:, :])
                nc.vector.tensor_add(t_fc[:, :], t_fc[:, :], t_ig[:, :])
                nc.scalar.activation(t_ig[:, :], t_fc[:, :], AF.Tanh)
                nc.vector.tensor_mul(t_ig[:, :], gt[:, 3 * H:4 * H], t_ig[:, :])
                mb = mbp.tile([BL, H], F32)
                nc.vector.tensor_sub(mb[:, :], t_fc[:, :], c_st[:, :])
                nc.vector.scalar_tensor_tensor(
                    c_st[:, :], mb[:, :], mk_sb[:, ds(i, 1)], c_st[:, :],
                    op0=mybir.AluOpType.mult, op1=mybir.AluOpType.add)
                nc.vector.tensor_sub(mb[:, :], t_ig[:, :], h_st[:, :])
                nc.vector.scalar_tensor_tensor(
                    h_st[:, :], mb[:, :], mk_sb[:, ds(i, 1)], h_st[:, :],
                    op0=mybir.AluOpType.mult, op1=mybir.AluOpType.add)
                nc.vector.tensor_copy(hbf[:, :], h_st[:, :])
                tp = ptp.tile([128, 8 * BL], BF16)
                for k in range(8):
                    nc.tensor.transpose(tp[:, ts(k, BL)],
                                        hbf[:, ts(k, 128)], id8_sb[:, :])
                nc.scalar.copy(hT[:, :], tp[:, :])
                nc.sync.dma_start(
                    oenc[ds(i, 1), :, :].rearrange("o b c -> (o b) c"),
                    h_st[:, :])

            nc.sync.dma_start(ohc[0, :, :], h_st[:, :])
            nc.sync.dma_start(ohc[1, :, :], c_st[:, :])

    _split_waits(nc, cap=1)
    return nc


_NC = None


def kernel(src_token_embeddings, padding_mask, W_fw, U_fw, b_fw,
           W_bw, U_bw, b_bw):
    global _NC
    if _NC is None:
        _NC = _build()
    nc = _NC

    x = np.asarray(src_token_embeddings, np.float32)
    keep = 1.0 - np.asarray(padding_mask, np.float32)  # 1 = keep
    bf = ml_dtypes.bfloat16
    id8 = np.eye(BL, dtype=bf)

    def waug(Wd, bd):
        out = np.zeros((HA, G4), np.float32)
        out[:H] = Wd
        out[H] = bd
        return out.astype(bf)

    W_a = {0: waug(W_fw, b_fw), 1: waug(W_bw, b_bw)}
    U_a = {0: np.asarray(U_fw, np.float32).astype(bf),
           1: np.asarray(U_bw, np.float32).astype(bf)}

    in_maps = []
    for core in range(8):
        d, q = core // 4, core % 4
        xc = x[q * BL:(q + 1) * BL]            # [8, S, H]
        mc = keep[q * BL:(q + 1) * BL]         # [8, S]
        if d == 1:
            xc = xc[:, ::-1]
            mc = mc[:, ::-1]
        xTa = np.zeros((HA, NB), np.float32)
        xTa[:H] = xc.transpose(2, 0, 1).reshape(H, NB)  # t = b*S + s
        xTa[H] = 1.0
        in_maps.append({
            "xT": np.ascontiguousarray(xTa.astype(bf)),
            "Wt": W_a[d], "Ut": U_a[d],
            "mk": np.ascontiguousarray(mc.astype(np.float32)),
            "id8": id8,
        })

    res = run_bass_kernel_spmd(nc, in_maps, core_ids=list(range(8)))

    enc = np.zeros((B, S, 2 * H), np.float32)
    h_fw = np.zeros((B, H), np.float32)
    c_fw = np.zeros((B, H), np.float32)
    h_bw = np.zeros((B, H), np.float32)
    c_bw = np.zeros((B, H), np.float32)
    for core in range(8):
        d, q = core // 4, core % 4
        o = res.results[core]["oenc"]          # [S, 8, H]
        hc = res.results[core]["ohc"]          # [2, 8, H]
        bsl = slice(q * BL, (q + 1) * BL)
        if d == 0:
            enc[bsl, :, :H] = o.transpose(1, 0, 2)
            h_fw[bsl], c_fw[bsl] = hc[0], hc[1]
        else:
            enc[bsl, :, H:] = o[::-1].transpose(1, 0, 2)
            h_bw[bsl], c_bw[bsl] = hc[0], hc[1]
    return enc, h_fw, c_fw, h_bw, c_bw
